# revision 14
# baseline (speedup 1.0000x reference)
"""Trainium2 Bass kernel for nn_MultiHeadedAttention_53626961658052.

Full-input contract: kernel(**inputs) takes the unsharded numpy inputs and
returns the full outputs (mean_x [4,2048,64], q [4,16,2048,64]) as a tuple,
matching the reference.

Sharding: 8 cores = 4 batches x 2 head-halves. Core c handles batch c//2 and
heads (c%2)*8 .. (c%2)*8+8. Per core:
  - k/v/wk are cast to bf16 in DRAM and their transposes come in via DMA
    transpose (16-bit xbar path), so only the q path (fp32r for output
    fidelity) uses PE transposes;
  - scores^T = k_h^T q_h per head as two K=64 matmuls row-packed into the
    128x128 PE array; exp on the scalar engine straight from PSUM (scale=1/8
    fused; max-subtraction skipped: scores are in [-10, 11]);
  - x^T = [v | 16]^T @ p^T with M=65 matmuls (ones column -> 16*rowsum,
    folding the /16 head-mean into the reciprocal);
  - x^T is transposed back on the PE and normalized/accumulated on DVE.

The kernel is scalar-engine(exp)-bound in steady state; the Q projection is
chopped up and threaded through the attention jc-loops so the PE slack
absorbs it without starving the scalar engine.
"""

import os
import numpy as np

import concourse.bass as bass
import concourse.mybir as mybir
import concourse.tile as tile
from concourse import bacc
from concourse.bass_utils import run_bass_kernel_spmd
from concourse.masks import make_identity
from contextlib import ExitStack

F32 = mybir.dt.float32
F32R = mybir.dt.float32r
BF16 = mybir.dt.bfloat16
Exp = mybir.ActivationFunctionType.Exp
MUL = mybir.AluOpType.mult
ADD = mybir.AluOpType.add

S = 2048
D = 1024
M = 512          # head-dim columns per core = 8 heads * 64
NHEAD = 8
NPAIR = 4
DK = 64

_built = None


def _build():
    nc = bacc.Bacc(None, target_bir_lowering=False)
    query = nc.dram_tensor("query", [S, D], F32, kind="ExternalInput")
    key = nc.dram_tensor("key", [S, D], F32, kind="ExternalInput")
    value = nc.dram_tensor("value", [DK, S], F32, kind="ExternalInput")
    wq = nc.dram_tensor("wq", [M, D], F32, kind="ExternalInput")
    wk = nc.dram_tensor("wk", [M, D], F32, kind="ExternalInput")
    bq = nc.dram_tensor("bq", [M], F32, kind="ExternalInput")
    bk = nc.dram_tensor("bk", [M], F32, kind="ExternalInput")
    qout = nc.dram_tensor("qout", [M, S], F32, kind="ExternalOutput")
    xout = nc.dram_tensor("xout", [S, DK], F32, kind="ExternalOutput")

    with tile.TileContext(nc) as tc, ExitStack() as ctx:
        const = ctx.enter_context(tc.tile_pool(name="const", bufs=1))
        dram = ctx.enter_context(tc.tile_pool(name="dram", bufs=1, space="DRAM"))

        ident_f = const.tile([128, 128], F32)
        make_identity(nc, ident_f)

        # bf16 DRAM staging for everything the 16-bit DMA-transpose can feed
        key_bf = dram.tile([S, D], BF16)
        wk_bf = dram.tile([M, D], BF16)
        # chunked so each [64, 128] transpose source is fully contiguous
        val_bf = dram.tile([16, DK, 128], BF16)
        for sc in range(4):
            nc.gpsimd.dma_start(out=key_bf[sc * 512:(sc + 1) * 512, :],
                                in_=key[sc * 512:(sc + 1) * 512, :])
        nc.gpsimd.dma_start(out=wk_bf, in_=wk[:, :])
        for jc in range(16):
            nc.gpsimd.dma_start(out=val_bf[jc, :, :],
                                in_=value[:, jc * 128:(jc + 1) * 128])

        # v^T with a 16.0 column appended -> matmul row 64 = 16*rowsum
        vplus = const.tile([128, 16, 65], BF16)
        nc.gpsimd.memset(vplus[:, :, 64:65], 16.0)
        with tc.tile_pool(name="vstg", bufs=2) as vstg_pool:
            for jc in range(16):
                vstg = vstg_pool.tile([128, DK], BF16, tag="vstg")
                nc.sync.dma_start(out=vstg, in_=val_bf[jc, :, :], transpose=True)
                nc.gpsimd.tensor_copy(vplus[:, jc, 0:DK], vstg)

        wqT = const.tile([128, 8, M], F32R)
        wkT = const.tile([128, 8, M], BF16)
        for dc in range(8):
            for wmc in range(4):
                nc.sync.dma_start(
                    out=wkT[:, dc, wmc * 128:(wmc + 1) * 128],
                    in_=wk_bf[wmc * 128:(wmc + 1) * 128, dc * 128:(dc + 1) * 128],
                    transpose=True)

        with tc.tile_pool(name="wstage", bufs=2) as wstage_pool, \
             tc.tile_pool(name="wps", bufs=2, space="PSUM") as wps_pool:
            for wmc in range(4):
                wstage = wstage_pool.tile([128, D], F32, tag="wstage")
                nc.sync.dma_start(out=wstage,
                                  in_=wq[wmc * 128:(wmc + 1) * 128, :])
                for dc in range(8):
                    wps = wps_pool.tile([128, 128], F32, tag="wps")
                    nc.tensor.transpose(wps, wstage[:, dc * 128:(dc + 1) * 128],
                                        ident_f)
                    nc.scalar.copy(wqT[:, dc, wmc * 128:(wmc + 1) * 128], wps)

        bqsb = const.tile([128, 4], F32)
        bksb = const.tile([128, 4], F32)
        for mc in range(4):
            nc.sync.dma_start(out=bqsb[:, mc:mc + 1],
                              in_=bq[mc * 128:(mc + 1) * 128].unsqueeze(1))
            nc.sync.dma_start(out=bksb[:, mc:mc + 1],
                              in_=bk[mc * 128:(mc + 1) * 128].unsqueeze(1))

        # persistent projection outputs (bf16) laid out per head-pair
        qT_pair = [const.tile([128, S], BF16, name=f"qTp{p}") for p in range(NPAIR)]
        kT_pair = [const.tile([128, S], BF16, name=f"kTp{p}") for p in range(NPAIR)]
        x_acc = const.tile([128, 16, DK], F32)

        # ---- K projection (prefix): kT tiles straight from DMA transpose ----
        with tc.tile_pool(name="kTd", bufs=4) as kTd_pool, \
             tc.tile_pool(name="kacc", bufs=1, space="PSUM") as kacc_pool:
            for sc in range(4):
                acc = kacc_pool.tile([128, 4, 512], F32, tag="kacc")
                for dc in range(8):
                    kT = kTd_pool.tile([128, 512], BF16, tag="kTd")
                    nc.sync.dma_start(
                        out=kT,
                        in_=key_bf[sc * 512:(sc + 1) * 512, dc * 128:(dc + 1) * 128],
                        transpose=True)
                    for mc in range(4):
                        nc.tensor.matmul(acc[:, mc, :],
                                         wkT[:, dc, mc * 128:(mc + 1) * 128], kT,
                                         start=(dc == 0), stop=(dc == 7))
                for mc in range(4):
                    if mc % 2 == 0:
                        nc.scalar.add(kT_pair[mc][:, sc * 512:(sc + 1) * 512],
                                      acc[:, mc, :], bksb[:, mc:mc + 1])
                    else:
                        nc.vector.tensor_scalar_add(
                            kT_pair[mc][:, sc * 512:(sc + 1) * 512],
                            acc[:, mc, :], bksb[:, mc:mc + 1])

        # ---- attention with Q projection threaded through ----
        # PSUM: sc 4 banks + xA/xB 2 banks + misc 2 banks = 8
        with tc.tile_pool(name="scps", bufs=2, space="PSUM") as sc_pool, \
             tc.tile_pool(name="xps", bufs=1, space="PSUM") as x_pool, \
             tc.tile_pool(name="misc", bufs=2, space="PSUM") as misc_pool, \
             tc.tile_pool(name="qin", bufs=5) as qin_pool, \
             tc.tile_pool(name="qTd", bufs=9) as qTd_pool, \
             tc.tile_pool(name="qsb", bufs=3) as qsb_pool, \
             tc.tile_pool(name="pT", bufs=3) as pT_pool, \
             tc.tile_pool(name="xTs", bufs=2) as xTs_pool, \
             tc.tile_pool(name="small", bufs=4) as small_pool:

            def q_load(sc):
                qins = []
                for sj in range(4):
                    qin = qin_pool.tile([128, D], F32, tag="qin")
                    nc.sync.dma_start(
                        out=qin,
                        in_=query[sc * 512 + sj * 128: sc * 512 + (sj + 1) * 128, :])
                    qins.append(qin)
                return qins

            def q_transpose_group(qins, dc, out_tiles):
                tpq = misc_pool.tile([128, 512], F32, tag="misc", name="tpq")
                for sj in range(4):
                    nc.tensor.transpose(
                        tpq[:, sj * 128:(sj + 1) * 128],
                        qins[sj][:, dc * 128:(dc + 1) * 128], ident_f)
                qTd = qTd_pool.tile([128, 512], F32R, tag="qTd", name="qTd")
                nc.vector.tensor_copy(qTd, tpq)
                out_tiles.append(qTd)

            def q_proj_mm(sc, mc, qTd_tiles, dc, state):
                if dc == 0:
                    state[mc] = misc_pool.tile([128, 512], F32, tag="misc",
                                               name="accq")
                nc.tensor.matmul(state[mc],
                                 wqT[:, dc, mc * 128:(mc + 1) * 128],
                                 qTd_tiles[dc],
                                 start=(dc == 0), stop=(dc == 7))

            def q_proj_fin(sc, mc, state, use_act=False):
                accq = state[mc]
                qsb = qsb_pool.tile([128, 512], F32, tag="qsb")
                if use_act:
                    nc.scalar.add(qsb, accq, bqsb[:, mc:mc + 1])
                else:
                    nc.vector.tensor_scalar_add(qsb, accq, bqsb[:, mc:mc + 1])
                nc.sync.dma_start(
                    out=qout[mc * 128:(mc + 1) * 128, sc * 512:(sc + 1) * 512],
                    in_=qsb)
                nc.vector.tensor_scalar_add(
                    qT_pair[mc][:, sc * 512:(sc + 1) * 512],
                    accq, bqsb[:, mc:mc + 1])

            def attention(p, ic, side):
                kT = kT_pair[p]
                qT = qT_pair[p]
                xA = x_pool.tile([65, 512], F32, tag="xA")
                xB = x_pool.tile([65, 512], F32, tag="xB")
                for jc in range(16):
                    scps = sc_pool.tile([128, 2, 512], F32, tag="sc")
                    nc.tensor.matmul(scps[:, 0, :],
                                     kT[0:64, jc * 128:(jc + 1) * 128],
                                     qT[0:64, ic * 512:(ic + 1) * 512],
                                     start=True, stop=True)
                    nc.tensor.matmul(scps[:, 1, :],
                                     kT[64:128, jc * 128:(jc + 1) * 128],
                                     qT[64:128, ic * 512:(ic + 1) * 512],
                                     start=True, stop=True)
                    pT = pT_pool.tile([128, 2, 512], BF16, tag="pT")
                    nc.scalar.activation(pT, scps, Exp, scale=0.125)
                    nc.tensor.matmul(xA, vplus[:, jc, :], pT[:, 0, :],
                                     start=(jc == 0), stop=(jc == 15))
                    nc.tensor.matmul(xB, vplus[:, jc, :], pT[:, 1, :],
                                     start=(jc == 0), stop=(jc == 15))
                    if side and jc < len(side) and side[jc] is not None:
                        side[jc]()
                for a, xps in ((0, xA), (1, xB)):
                    xTs = xTs_pool.tile([65, 512], F32, tag="xTs")
                    nc.vector.tensor_copy(xTs, xps)
                    for t in range(4):
                        xp = misc_pool.tile([128, 512], F32, tag="misc", name="xp")
                        nc.tensor.transpose(xp[:, 0:65],
                                            xTs[:, t * 128:(t + 1) * 128],
                                            ident_f[0:65, 0:65])
                        r = small_pool.tile([128, 1], F32, tag="r")
                        nc.vector.reciprocal(r, xp[:, DK:DK + 1])
                        tg = ic * 4 + t
                        if p == 0 and a == 0:
                            nc.vector.tensor_scalar_mul(x_acc[:, tg, :],
                                                        xp[:, 0:DK], r)
                        else:
                            nc.vector.scalar_tensor_tensor(
                                out=x_acc[:, tg, :], in0=xp[:, 0:DK], scalar=r,
                                in1=x_acc[:, tg, :], op0=MUL, op1=ADD)

            # sc0 Q work runs as prefix (ACT is idle until first scores land)
            qins0 = q_load(0)
            tiles0 = []
            for dc in range(8):
                q_transpose_group(qins0, dc, tiles0)
            st0 = {}
            for mc in range(4):
                for dc in range(8):
                    q_proj_mm(0, mc, tiles0, dc, st0)
                q_proj_fin(0, mc, st0, use_act=True)

            tiles = {0: tiles0}
            for ic in range(4):
                nsc = ic + 1
                if nsc < 4:
                    qins = q_load(nsc)
                    tiles[nsc] = []
                    st = {}
                for p in range(NPAIR):
                    side = [None] * 16
                    if nsc < 4 and not os.environ.get("KERNEL_NO_INTERLEAVE"):
                        if p == 0:
                            # 8 transpose groups, every other jc
                            for g in range(8):
                                side[2 * g] = (lambda g=g, q=qins, t=tiles[nsc]:
                                               q_transpose_group(q, g, t))
                        elif p == 1:
                            for dc in range(8):
                                side[2 * dc] = (lambda dc=dc, t=tiles[nsc], s=st:
                                                q_proj_mm(nsc, 0, t, dc, s))
                            side[15] = (lambda s=st: q_proj_fin(nsc, 0, s))
                        elif p == 2:
                            for dc in range(8):
                                side[2 * dc] = (lambda dc=dc, t=tiles[nsc], s=st:
                                                q_proj_mm(nsc, 1, t, dc, s))
                            side[15] = (lambda s=st: q_proj_fin(nsc, 1, s))
                        elif p == 3:
                            for dc in range(8):
                                side[dc] = (lambda dc=dc, t=tiles[nsc], s=st:
                                            q_proj_mm(nsc, 2, t, dc, s))
                                side[8 + dc] = (lambda dc=dc, t=tiles[nsc], s=st:
                                                q_proj_mm(nsc, 3, t, dc, s))
                            # finalizers after the loop, before the epilogue
                    attention(p, ic, side)
                    if nsc < 4 and os.environ.get("KERNEL_NO_INTERLEAVE"):
                        if p == 0:
                            for g in range(8):
                                q_transpose_group(qins, g, tiles[nsc])
                        elif p == 1:
                            for mc in (0, 1):
                                for dc in range(8):
                                    q_proj_mm(nsc, mc, tiles[nsc], dc, st)
                                q_proj_fin(nsc, mc, st)
                        elif p == 2:
                            for mc in (2, 3):
                                for dc in range(8):
                                    q_proj_mm(nsc, mc, tiles[nsc], dc, st)
                                q_proj_fin(nsc, mc, st)
                    elif nsc < 4 and p == 3:
                        q_proj_fin(nsc, 2, st)
                        q_proj_fin(nsc, 3, st)

        nc.sync.dma_start(out=xout[:, :].rearrange("(t p) e -> p t e", p=128),
                          in_=x_acc)

    nc.finalize()
    return nc


def _get_built():
    global _built
    if _built is None:
        _built = _build()
    return _built


def _make_in_maps(inputs):
    query = np.asarray(inputs["query"], dtype=np.float32)
    key = np.asarray(inputs["key"], dtype=np.float32)
    value = np.asarray(inputs["value"], dtype=np.float32)
    Wq = np.asarray(inputs["Wq"], dtype=np.float32)
    bq = np.asarray(inputs["bq"], dtype=np.float32)
    Wk = np.asarray(inputs["Wk"], dtype=np.float32)
    bk = np.asarray(inputs["bk"], dtype=np.float32)
    in_maps = []
    for c in range(8):
        b, hh = c // 2, c % 2
        sl = slice(hh * M, (hh + 1) * M)
        in_maps.append({
            "query": query[b],
            "key": key[b],
            "value": value[b],
            "wq": np.ascontiguousarray(Wq[sl]),
            "wk": np.ascontiguousarray(Wk[sl]),
            "bq": np.ascontiguousarray(bq[sl]),
            "bk": np.ascontiguousarray(bk[sl]),
        })
    return in_maps


def kernel(query, key, value, Wq, bq, Wk, bk):
    nc = _get_built()
    in_maps = _make_in_maps(dict(query=query, key=key, value=value,
                                 Wq=Wq, bq=bq, Wk=Wk, bk=bk))
    res = run_bass_kernel_spmd(nc, in_maps, list(range(8)))

    B = np.asarray(query).shape[0]
    H = 16
    q_full = np.empty((B, H, S, DK), dtype=np.float32)
    mean_x = np.empty((B, S, DK), dtype=np.float32)
    for c in range(8):
        b, hh = c // 2, c % 2
        r = res.results[c]
        q_full[b, hh * NHEAD:(hh + 1) * NHEAD] = (
            r["qout"].reshape(NHEAD, DK, S).transpose(0, 2, 1))
        if hh == 0:
            mean_x[b] = r["xout"]
        else:
            mean_x[b] += r["xout"]
    return mean_x, q_full


# revision 16
# speedup vs baseline: 1.4012x; 1.4012x over previous
"""Trainium2 Bass kernel for nn_MultiHeadedAttention_53626961658052.

Full-input contract: kernel(**inputs) takes the unsharded numpy inputs and
returns the full outputs (mean_x [4,2048,64], q [4,16,2048,64]) as a tuple,
matching the reference.

Sharding: 8 cores = 4 batches x 2 head-halves. Core c handles batch c//2 and
heads (c%2)*8 .. (c%2)*8+8. Per core:
  - k/v/wk are cast to bf16 in DRAM and their transposes come in via DMA
    transpose (16-bit xbar path), so only the q path (fp32r for output
    fidelity) uses PE transposes;
  - scores^T = k_h^T q_h per head as two K=64 matmuls row-packed into the
    128x128 PE array; exp on the scalar engine straight from PSUM (scale=1/8
    fused; max-subtraction skipped: scores are in [-10, 11]);
  - x^T = [v | 16]^T @ p^T with M=65 matmuls (ones column -> 16*rowsum,
    folding the /16 head-mean into the reciprocal);
  - x^T is transposed back on the PE and normalized/accumulated on DVE.

The kernel is scalar-engine(exp)-bound in steady state; the Q projection is
chopped up and threaded through the attention jc-loops so the PE slack
absorbs it without starving the scalar engine.
"""

import os
import numpy as np

import concourse.bass as bass
import concourse.mybir as mybir
import concourse.tile as tile
from concourse import bacc
from concourse.bass_utils import run_bass_kernel_spmd
from concourse.masks import make_identity
from contextlib import ExitStack

F32 = mybir.dt.float32
F32R = mybir.dt.float32r
BF16 = mybir.dt.bfloat16
Exp = mybir.ActivationFunctionType.Exp
MUL = mybir.AluOpType.mult
ADD = mybir.AluOpType.add

S = 2048
D = 1024
M = 512          # head-dim columns per core = 8 heads * 64
NHEAD = 8
NPAIR = 4
DK = 64

_built = None


def _build():
    nc = bacc.Bacc(None, target_bir_lowering=False)
    query = nc.dram_tensor("query", [S, D], F32, kind="ExternalInput")
    key = nc.dram_tensor("key", [S, D], F32, kind="ExternalInput")
    value = nc.dram_tensor("value", [DK, S], F32, kind="ExternalInput")
    wq = nc.dram_tensor("wq", [M, D], F32, kind="ExternalInput")
    wk = nc.dram_tensor("wk", [M, D], F32, kind="ExternalInput")
    bq = nc.dram_tensor("bq", [M], F32, kind="ExternalInput")
    bk = nc.dram_tensor("bk", [M], F32, kind="ExternalInput")
    qout = nc.dram_tensor("qout", [M, S], F32, kind="ExternalOutput")
    xout = nc.dram_tensor("xout", [S, DK], F32, kind="ExternalOutput")

    with tile.TileContext(nc) as tc, ExitStack() as ctx:
        const = ctx.enter_context(tc.tile_pool(name="const", bufs=1))
        dram = ctx.enter_context(tc.tile_pool(name="dram", bufs=1, space="DRAM"))

        ident_f = const.tile([128, 128], F32)
        make_identity(nc, ident_f)
        ident_b = const.tile([128, 128], BF16)
        make_identity(nc, ident_b)

        # v^T with a 16.0 column appended -> matmul row 64 = 16*rowsum
        vplus = const.tile([128, 16, 65], BF16)
        nc.gpsimd.memset(vplus[:, :, 64:65], 16.0)
        vstage = const.tile([DK, S], F32)
        nc.sync.dma_start(out=vstage, in_=value[:, :])
        with tc.tile_pool(name="vps", bufs=2, space="PSUM") as vps_pool:
            for jc in range(16):
                vps = vps_pool.tile([128, DK], F32, tag="vps")
                nc.tensor.transpose(vps, vstage[:, jc * 128:(jc + 1) * 128],
                                    ident_f[0:DK, 0:DK])
                nc.scalar.copy(vplus[:, jc, 0:DK], vps)

        wqT = const.tile([128, 8, M], F32R)
        wkT = const.tile([128, 8, M], BF16)
        with tc.tile_pool(name="wstage", bufs=2) as wstage_pool, \
             tc.tile_pool(name="wps", bufs=2, space="PSUM") as wps_pool:
            for wdram, wT in ((wq, wqT), (wk, wkT)):
                for wmc in range(4):
                    wstage = wstage_pool.tile([128, D], F32, tag="wstage")
                    nc.sync.dma_start(out=wstage,
                                      in_=wdram[wmc * 128:(wmc + 1) * 128, :])
                    for dc in range(8):
                        wps = wps_pool.tile([128, 128], F32, tag="wps")
                        nc.tensor.transpose(wps, wstage[:, dc * 128:(dc + 1) * 128],
                                            ident_f)
                        nc.scalar.copy(wT[:, dc, wmc * 128:(wmc + 1) * 128], wps)

        bqsb = const.tile([128, 4], F32)
        bksb = const.tile([128, 4], F32)
        for mc in range(4):
            nc.sync.dma_start(out=bqsb[:, mc:mc + 1],
                              in_=bq[mc * 128:(mc + 1) * 128].unsqueeze(1))
            nc.sync.dma_start(out=bksb[:, mc:mc + 1],
                              in_=bk[mc * 128:(mc + 1) * 128].unsqueeze(1))

        # persistent projection outputs (bf16) laid out per head-pair
        qT_pair = [const.tile([128, S], BF16, name=f"qTp{p}") for p in range(NPAIR)]
        kT_pair = [const.tile([128, S], BF16, name=f"kTp{p}") for p in range(NPAIR)]
        x_acc = const.tile([128, 16, DK], F32)

        # ---- K projection (prefix): bf16 end-to-end via PE transposes ----
        with tc.tile_pool(name="kin", bufs=9) as kin_pool, \
             tc.tile_pool(name="kTd", bufs=3) as kTd_pool, \
             tc.tile_pool(name="ktp", bufs=3, space="PSUM") as ktp_pool, \
             tc.tile_pool(name="kacc", bufs=1, space="PSUM") as kacc_pool:
            for sc in range(4):
                kins = []
                for sj in range(4):
                    kin = kin_pool.tile([128, D], BF16, tag="kin")
                    nc.gpsimd.dma_start(
                        out=kin,
                        in_=key[sc * 512 + sj * 128: sc * 512 + (sj + 1) * 128, :])
                    kins.append(kin)
                acc = kacc_pool.tile([128, 4, 512], F32, tag="kacc")
                for dc in range(8):
                    tp = ktp_pool.tile([128, 512], BF16, tag="ktp")
                    for sj in range(4):
                        nc.tensor.transpose(
                            tp[:, sj * 128:(sj + 1) * 128],
                            kins[sj][:, dc * 128:(dc + 1) * 128], ident_b)
                    kT = kTd_pool.tile([128, 512], BF16, tag="kTd")
                    nc.vector.tensor_copy(kT, tp)
                    for mc in range(4):
                        nc.tensor.matmul(acc[:, mc, :],
                                         wkT[:, dc, mc * 128:(mc + 1) * 128], kT,
                                         start=(dc == 0), stop=(dc == 7))
                for mc in range(4):
                    if mc % 2 == 0:
                        nc.scalar.add(kT_pair[mc][:, sc * 512:(sc + 1) * 512],
                                      acc[:, mc, :], bksb[:, mc:mc + 1])
                    else:
                        nc.vector.tensor_scalar_add(
                            kT_pair[mc][:, sc * 512:(sc + 1) * 512],
                            acc[:, mc, :], bksb[:, mc:mc + 1])

        # ---- attention with Q projection threaded through ----
        # PSUM: sc 4 banks + xA/xB 2 banks + misc 2 banks = 8
        with tc.tile_pool(name="scps", bufs=2, space="PSUM") as sc_pool, \
             tc.tile_pool(name="xps", bufs=1, space="PSUM") as x_pool, \
             tc.tile_pool(name="misc", bufs=2, space="PSUM") as misc_pool, \
             tc.tile_pool(name="qin", bufs=5) as qin_pool, \
             tc.tile_pool(name="qTd", bufs=9) as qTd_pool, \
             tc.tile_pool(name="qsb", bufs=3) as qsb_pool, \
             tc.tile_pool(name="pT", bufs=3) as pT_pool, \
             tc.tile_pool(name="xTs", bufs=2) as xTs_pool, \
             tc.tile_pool(name="small", bufs=4) as small_pool:

            def q_load(sc):
                qins = []
                for sj in range(4):
                    qin = qin_pool.tile([128, D], F32, tag="qin")
                    nc.sync.dma_start(
                        out=qin,
                        in_=query[sc * 512 + sj * 128: sc * 512 + (sj + 1) * 128, :])
                    qins.append(qin)
                return qins

            def q_transpose_group(qins, dc, out_tiles):
                tpq = misc_pool.tile([128, 512], F32, tag="misc", name="tpq")
                for sj in range(4):
                    nc.tensor.transpose(
                        tpq[:, sj * 128:(sj + 1) * 128],
                        qins[sj][:, dc * 128:(dc + 1) * 128], ident_f)
                qTd = qTd_pool.tile([128, 512], F32R, tag="qTd", name="qTd")
                nc.vector.tensor_copy(qTd, tpq)
                out_tiles.append(qTd)

            def q_proj_mm(sc, mc, qTd_tiles, dc, state):
                if dc == 0:
                    state[mc] = misc_pool.tile([128, 512], F32, tag="misc",
                                               name="accq")
                nc.tensor.matmul(state[mc],
                                 wqT[:, dc, mc * 128:(mc + 1) * 128],
                                 qTd_tiles[dc],
                                 start=(dc == 0), stop=(dc == 7))

            def q_proj_fin(sc, mc, state, use_act=False):
                accq = state[mc]
                qsb = qsb_pool.tile([128, 512], F32, tag="qsb")
                if use_act:
                    nc.scalar.add(qsb, accq, bqsb[:, mc:mc + 1])
                else:
                    nc.vector.tensor_scalar_add(qsb, accq, bqsb[:, mc:mc + 1])
                nc.sync.dma_start(
                    out=qout[mc * 128:(mc + 1) * 128, sc * 512:(sc + 1) * 512],
                    in_=qsb)
                nc.vector.tensor_scalar_add(
                    qT_pair[mc][:, sc * 512:(sc + 1) * 512],
                    accq, bqsb[:, mc:mc + 1])

            def attention(p, ic, side):
                kT = kT_pair[p]
                qT = qT_pair[p]
                xA = x_pool.tile([65, 512], F32, tag="xA")
                xB = x_pool.tile([65, 512], F32, tag="xB")
                for jc in range(16):
                    scps = sc_pool.tile([128, 2, 512], F32, tag="sc")
                    nc.tensor.matmul(scps[:, 0, :],
                                     kT[0:64, jc * 128:(jc + 1) * 128],
                                     qT[0:64, ic * 512:(ic + 1) * 512],
                                     start=True, stop=True)
                    nc.tensor.matmul(scps[:, 1, :],
                                     kT[64:128, jc * 128:(jc + 1) * 128],
                                     qT[64:128, ic * 512:(ic + 1) * 512],
                                     start=True, stop=True)
                    pT = pT_pool.tile([128, 2, 512], BF16, tag="pT")
                    nc.scalar.activation(pT, scps, Exp, scale=0.125)
                    nc.tensor.matmul(xA, vplus[:, jc, :], pT[:, 0, :],
                                     start=(jc == 0), stop=(jc == 15))
                    nc.tensor.matmul(xB, vplus[:, jc, :], pT[:, 1, :],
                                     start=(jc == 0), stop=(jc == 15))
                    if side and jc < len(side) and side[jc] is not None:
                        side[jc]()
                for a, xps in ((0, xA), (1, xB)):
                    xTs = xTs_pool.tile([65, 512], F32, tag="xTs")
                    nc.vector.tensor_copy(xTs, xps)
                    for t in range(4):
                        xp = misc_pool.tile([128, 512], F32, tag="misc", name="xp")
                        nc.tensor.transpose(xp[:, 0:65],
                                            xTs[:, t * 128:(t + 1) * 128],
                                            ident_f[0:65, 0:65])
                        r = small_pool.tile([128, 1], F32, tag="r")
                        nc.vector.reciprocal(r, xp[:, DK:DK + 1])
                        tg = ic * 4 + t
                        if p == 0 and a == 0:
                            nc.vector.tensor_scalar_mul(x_acc[:, tg, :],
                                                        xp[:, 0:DK], r)
                        else:
                            nc.vector.scalar_tensor_tensor(
                                out=x_acc[:, tg, :], in0=xp[:, 0:DK], scalar=r,
                                in1=x_acc[:, tg, :], op0=MUL, op1=ADD)

            # sc0 Q work runs as prefix (ACT is idle until first scores land)
            qins0 = q_load(0)
            tiles0 = []
            for dc in range(8):
                q_transpose_group(qins0, dc, tiles0)
            st0 = {}
            for mc in range(4):
                for dc in range(8):
                    q_proj_mm(0, mc, tiles0, dc, st0)
                q_proj_fin(0, mc, st0, use_act=True)

            tiles = {0: tiles0}
            for ic in range(4):
                nsc = ic + 1
                if nsc < 4:
                    qins = q_load(nsc)
                    tiles[nsc] = []
                    st = {}
                for p in range(NPAIR):
                    side = [None] * 16
                    if nsc < 4 and not os.environ.get("KERNEL_NO_INTERLEAVE"):
                        if p == 0:
                            # 8 transpose groups, every other jc
                            for g in range(8):
                                side[2 * g] = (lambda g=g, q=qins, t=tiles[nsc]:
                                               q_transpose_group(q, g, t))
                        elif p == 1:
                            for dc in range(8):
                                side[2 * dc] = (lambda dc=dc, t=tiles[nsc], s=st:
                                                q_proj_mm(nsc, 0, t, dc, s))
                            side[15] = (lambda s=st: q_proj_fin(nsc, 0, s))
                        elif p == 2:
                            for dc in range(8):
                                side[2 * dc] = (lambda dc=dc, t=tiles[nsc], s=st:
                                                q_proj_mm(nsc, 1, t, dc, s))
                            side[15] = (lambda s=st: q_proj_fin(nsc, 1, s))
                        elif p == 3:
                            for dc in range(8):
                                side[dc] = (lambda dc=dc, t=tiles[nsc], s=st:
                                            q_proj_mm(nsc, 2, t, dc, s))
                                side[8 + dc] = (lambda dc=dc, t=tiles[nsc], s=st:
                                                q_proj_mm(nsc, 3, t, dc, s))
                            # finalizers after the loop, before the epilogue
                    attention(p, ic, side)
                    if nsc < 4 and os.environ.get("KERNEL_NO_INTERLEAVE"):
                        if p == 0:
                            for g in range(8):
                                q_transpose_group(qins, g, tiles[nsc])
                        elif p == 1:
                            for mc in (0, 1):
                                for dc in range(8):
                                    q_proj_mm(nsc, mc, tiles[nsc], dc, st)
                                q_proj_fin(nsc, mc, st)
                        elif p == 2:
                            for mc in (2, 3):
                                for dc in range(8):
                                    q_proj_mm(nsc, mc, tiles[nsc], dc, st)
                                q_proj_fin(nsc, mc, st)
                    elif nsc < 4 and p == 3:
                        q_proj_fin(nsc, 2, st)
                        q_proj_fin(nsc, 3, st)

        nc.sync.dma_start(out=xout[:, :].rearrange("(t p) e -> p t e", p=128),
                          in_=x_acc)

    nc.finalize()
    return nc


def _get_built():
    global _built
    if _built is None:
        _built = _build()
    return _built


def _make_in_maps(inputs):
    query = np.asarray(inputs["query"], dtype=np.float32)
    key = np.asarray(inputs["key"], dtype=np.float32)
    value = np.asarray(inputs["value"], dtype=np.float32)
    Wq = np.asarray(inputs["Wq"], dtype=np.float32)
    bq = np.asarray(inputs["bq"], dtype=np.float32)
    Wk = np.asarray(inputs["Wk"], dtype=np.float32)
    bk = np.asarray(inputs["bk"], dtype=np.float32)
    in_maps = []
    for c in range(8):
        b, hh = c // 2, c % 2
        sl = slice(hh * M, (hh + 1) * M)
        in_maps.append({
            "query": query[b],
            "key": key[b],
            "value": value[b],
            "wq": np.ascontiguousarray(Wq[sl]),
            "wk": np.ascontiguousarray(Wk[sl]),
            "bq": np.ascontiguousarray(bq[sl]),
            "bk": np.ascontiguousarray(bk[sl]),
        })
    return in_maps


def kernel(query, key, value, Wq, bq, Wk, bk):
    nc = _get_built()
    in_maps = _make_in_maps(dict(query=query, key=key, value=value,
                                 Wq=Wq, bq=bq, Wk=Wk, bk=bk))
    res = run_bass_kernel_spmd(nc, in_maps, list(range(8)))

    B = np.asarray(query).shape[0]
    H = 16
    q_full = np.empty((B, H, S, DK), dtype=np.float32)
    mean_x = np.empty((B, S, DK), dtype=np.float32)
    for c in range(8):
        b, hh = c // 2, c % 2
        r = res.results[c]
        q_full[b, hh * NHEAD:(hh + 1) * NHEAD] = (
            r["qout"].reshape(NHEAD, DK, S).transpose(0, 2, 1))
        if hh == 0:
            mean_x[b] = r["xout"]
        else:
            mean_x[b] += r["xout"]
    return mean_x, q_full
